# revision 1
# baseline (speedup 1.0000x reference)
"""Trainium2 Bass kernel for nn_AdaptiveBoundaryRefinement_45861660787095.

Self-contained: takes FULL inputs (B=16,M=128,T=12000), shards batch across 8
NeuronCores (2 samples/core), runs a Bass/Tile kernel per core, gathers.

v2 design (validated numerically vs reference by numpy+bf16 prototype,
rel err ~5.5e-3 < 2e-2 gate):
- mel is cast f32->bf16 DURING the DMA (SWDGE/gpsimd queue), halving SBUF
  write traffic and enabling DVE 4x (2x_1p+2x_2p) bf16 modes downstream.
- Per-chunk M-reductions (S=col sums, Q=sum sq, D=cross dot) via PE matmuls
  with 32-wide one-hot lhsT slices (32-cycle weight loads, not 128): out is
  a [32, N] PSUM group at partition {0,32,64,96}; accumulate start/stop per
  (tensor, group).
- Sample 0 lives at partitions 0..46, sample 1 at 64..110 (chunk rows), so
  every per-sample compute op has a legal start partition and both samples
  share [0:111] precompute instructions.
- Per-sample PSUM accumulators: sample 0's tail overlaps sample 1's stream
  (tail-0 ops are interleaved into the emission between sample-1 tiles).
- Tail is mostly bf16 (DVE 4x): cos = D*rsqrt(Qa*Qb+eps) (no sqrt table:
  everything uses the reciprocal_sqrt_and_small ACT set, loaded once via a
  dummy Rsqrt at kernel start), consH = w0*cos + w1*spec_sim*validC,
  5-window sums via tensor_reduce views, closed-form 5-iteration refinement
  via predicated copies (same algebra as validated baseline).
- The batch-global early-stop of the reference is a mathematical no-op.
"""

import os
import sys

import numpy as np

_TRN_REPO = "/opt/trn_rl_repo"
if _TRN_REPO not in sys.path:
    sys.path.insert(0, _TRN_REPO)

import concourse.bass as bass
import concourse.bacc as bacc
import concourse.mybir as mybir
import concourse.tile as tile
from concourse.bass_utils import run_bass_kernel_spmd

F32 = mybir.dt.float32
F32R = mybir.dt.float32r
BF16 = mybir.dt.bfloat16
ALU = mybir.AluOpType
ACTF = mybir.ActivationFunctionType
AX = mybir.AxisListType

B, M, T = 16, 128, 12000
NCORES = 8
BPC = B // NCORES            # samples per core = 2
CH = 256                     # chunk width
NCH = (T + CH - 1) // CH     # 47 chunks per sample
EPS2 = 1e-12
GRAD_THRESH = 0.15
LASTW = T - CH * (NCH - 1)   # 224 real cols in the last chunk
NGS = [1, 4, 8, 8, 8, 8, 8, 2]  # chunks per mel tile (small ramp/tail tiles)
# mel DMA queue per tile: one queue sustains only ~220 GB/s read-side, so
# split across SWDGE (gpsimd, bf16-cast) and HWDGE (sync, f32 + ACT cast)
QSYNC = set()  # two-queue split regressed on HW; single SWDGE queue
SBASE = (0, 64)              # partition base of each sample's chunk rows
NROWS = SBASE[1] + NCH       # 111


def _softmax_f32(x):
    x = np.asarray(x, np.float32)
    m = np.max(x).astype(np.float32)
    e = np.exp((x - m).astype(np.float32)).astype(np.float32)
    return (e / e.sum(dtype=np.float32).astype(np.float32)).astype(np.float32)


def _const_masks(w0, w1):
    import ml_dtypes

    bf = ml_dtypes.bfloat16
    # w1vC [NROWS, 260]: w1 where col maps to t in [0, T), else 0
    # (col h of chunk p -> t = 256p - 2 + h).  Also t=0 keeps w1 (spec_sim=1
    # there via the specH shift trick).
    w1vC = np.full((NROWS, 260), np.float32(w1), np.float32)
    for r0 in SBASE:
        w1vC[r0, 0:2] = 0.0                       # t < 0
        w1vC[r0 + NCH - 1, LASTW + 2 : 260] = 0.0  # t >= T
    # SMb [NROWS, 256]: 0.2 interior, 0.25 at t=1 and t=T-2 (win4 edges)
    SM = np.full((NROWS, 256), np.float32(0.2), np.float32)
    for r0 in SBASE:
        SM[r0, 1] = 0.25
        SM[r0 + NCH - 1, LASTW - 2] = 0.25
    # gate01 [NROWS, 256]: 1 for t in [1, T-2], else 0
    gate = np.ones((NROWS, 256), np.float32)
    for r0 in SBASE:
        gate[r0, 0] = 0.0
        gate[r0 + NCH - 1, LASTW - 1 : 256] = 0.0
    # wzb one-hot bank [128, 257]: col 128 is ones; slice [128-j : 256-j]
    # puts the hot column at position j (j in 0..127).
    wzb = np.zeros((128, 257), bf)
    wzb[:, 128] = 1.0
    wz32 = np.zeros((128, 257), np.float32)
    wz32[:, 128] = 1.0
    # indB [128, 2*NROWS]: cols [0:NROWS] sample-0 block-indicator lhsT,
    # cols [NROWS:2*NROWS] sample-1.  out[i,:] = sum_{p in sample} W[p,:]
    # lands on every row i of that sample -> per-partition broadcast sums.
    indB = np.zeros((128, 2 * NROWS), bf)
    indB[0:NCH, 0:NCH] = 1.0
    indB[SBASE[1] : SBASE[1] + NCH, NROWS + SBASE[1] : NROWS + SBASE[1] + NCH] = 1.0
    return w1vC.astype(bf), SM.astype(bf), gate.astype(bf), wzb, indB, wz32


def build_nc(w0, w1, w2):
    nc = bacc.Bacc("TRN2", target_bir_lowering=False, debug=False)
    mel = nc.dram_tensor("mel_features", [BPC, M, T], F32, kind="ExternalInput")
    spec = nc.dram_tensor("spectral_features", [BPC, T], F32, kind="ExternalInput")
    init = nc.dram_tensor("initial_boundaries", [BPC, T], F32, kind="ExternalInput")
    out = nc.dram_tensor("out", [BPC, T], F32, kind="ExternalOutput")
    KDBG = bool(os.environ.get("KDBG"))
    dbg = {}
    if KDBG:
        for nm, w, dt in (
            ("dbg_W", 256, BF16), ("dbg_thr", 2, F32), ("dbg_local", 256, BF16),
            ("dbg_consH", 260, BF16), ("dbg_A", 256, BF16), ("dbg_Sb", 260, BF16),
            ("dbg_Qs", 261, BF16), ("dbg_cos", 260, BF16), ("dbg_t1sv", 260, BF16),
            ("dbg_Ds", 260, F32), ("dbg_den2", 260, BF16), ("dbg_sd", 260, F32),
            ("dbg_rs", 260, F32),
        ):
            dbg[nm] = nc.dram_tensor(nm, [BPC, NROWS, w], dt, kind="ExternalOutput")

    w1vC_np, SM_np, gate_np, wzb_np, indB_np, wz32_np = _const_masks(w0, w1)
    w1vC_d = nc.inline_tensor(w1vC_np, name="w1vC")
    wzb_d = nc.inline_tensor(wzb_np, name="wzbc")
    indB_d = nc.inline_tensor(indB_np, name="indB")
    SM_d = nc.inline_tensor(SM_np, name="SMb")
    gate_d = nc.inline_tensor(gate_np, name="gate01")


    th2 = float(np.float32(GRAD_THRESH) * np.float32(GRAD_THRESH))
    SMSC = 0.2 / 128.0

    with tile.TileContext(nc) as tc:
        with (
            tc.tile_pool(name="mel", bufs=6) as pmel,
            tc.tile_pool(name="mel32", bufs=4) as pmel32,
            tc.tile_pool(name="sq", bufs=4) as psq,
            tc.tile_pool(name="cross", bufs=4) as pcross,
            tc.tile_pool(name="stat", bufs=1) as pstat,
            tc.tile_pool(name="ps", bufs=1, space="PSUM") as pps,
            tc.tile_pool(name="ps2", bufs=1, space="PSUM") as pps2,
        ):
            # ---------------- startup constants ----------------
            # dummy Sqrt first: forces the sqrt_and_others table
            # (square/abs/copy/identity/sqrt) once, at t~0 — no mid-kernel
            # ACT table reload.
            dummy = pstat.tile([1, 1], F32)
            nc.vector.memset(dummy, 1.0)
            nc.scalar.activation(out=dummy, in_=dummy, func=ACTF.Sqrt)
            epsT = pstat.tile([128, 1], F32)
            nc.vector.memset(epsT, float(EPS2))

            WZb = pstat.tile([128, 257], BF16)
            nc.scalar.dma_start(out=WZb, in_=wzb_d[:, :])
            indB = pstat.tile([128, 2 * NROWS], BF16)
            nc.scalar.dma_start(out=indB, in_=indB_d[:, :])

            # small inputs go on the sync (SP/HWDGE) queue
            specH = pstat.tile([NROWS, 261], F32)  # spec at t = 256p-3+h
            nc.vector.memset(specH, 0.0)
            r = pstat.tile([NROWS, 256], F32)
            nc.vector.memset(r, 0.0)

            def _dma_rows(eng, dst, src_1d, row_lo, row_hi, col_off, width, t_base):
                ap = bass.AP(
                    tensor=src_1d.tensor,
                    offset=src_1d.offset + t_base,
                    ap=[[256, row_hi - row_lo], [1, width]],
                )
                eng.dma_start(
                    out=dst[row_lo:row_hi, col_off : col_off + width], in_=ap
                )

            def emit_smalls(b):
                r0 = SBASE[b]
                sp = spec[b]
                _dma_rows(nc.sync, specH, sp, r0, r0 + 1, 3, 258, 0)
                _dma_rows(nc.sync, specH, sp, r0, r0 + 1, 2, 1, 0)
                _dma_rows(nc.sync, specH, sp, r0 + 1, r0 + 46, 0, 261, 256 - 3)
                _dma_rows(nc.sync, specH, sp, r0 + 46, r0 + 47, 0, 227,
                          256 * 46 - 3)
                ini = init[b]
                _dma_rows(nc.sync, r, ini, r0, r0 + 46, 0, 256, 0)
                _dma_rows(nc.sync, r, ini, r0 + 46, r0 + 47, 0, LASTW, 256 * 46)


            w1vC = pstat.tile([NROWS, 260], BF16)
            nc.scalar.dma_start(out=w1vC, in_=w1vC_d[:, :])
            SMb = pstat.tile([NROWS, 256], BF16)
            nc.scalar.dma_start(out=SMb, in_=SM_d[:, :])
            gate01 = pstat.tile([NROWS, 256], BF16)
            nc.scalar.dma_start(out=gate01, in_=gate_d[:, :])

            emit_smalls(0)
            emit_smalls(1)

            # persistent first/last mel tiles (bf16), pads zeroed once
            W_FIRST = NGS[0] * CH + 6
            W_LAST = NGS[-1] * CH + 6
            T0_LAST = (NCH - NGS[-1]) * CH
            LASTREAL = T - (T0_LAST - 3)
            melt_firsts, melt_lasts = [], []
            for bb in range(BPC):
                mf = pstat.tile([128, W_FIRST], BF16, name=f"mf{bb}")
                nc.vector.memset(mf[:, 0:3], 0.0)
                melt_firsts.append(mf)
                ml = pstat.tile([128, W_LAST], BF16, name=f"ml{bb}")
                nc.vector.memset(ml[:, LASTREAL:W_LAST], 0.0)
                melt_lasts.append(ml)

            # ---------------- PSUM ----------------
            psS = [pps.tile([128, 512], F32, name=f"psS{b}") for b in range(BPC)]
            psQ = [pps.tile([128, 512], F32, name=f"psQ{b}") for b in range(BPC)]
            psD = [pps.tile([128, 512], F32, name=f"psD{b}") for b in range(BPC)]
            # widths used: S 260 (t=256p-2+h), Q 261 (t=256p-3+h), D 260
            # two extra banks hold the tiny temporal/broadcast outputs:
            # psT1_b=[64b,0:256], psT2_b=[64b,256:512], psB_b=[64b:64b+47,0:2]
            psTB = pps2.tile([128, 512], F32)
            psBB = pps2.tile([128, 512], F32)

            # accumulation-group bookkeeping: (tensor_id, sample, group32) ->
            # [n_seen, n_total]
            GROUPS = {}
            for b in range(BPC):
                for ti in range(3):
                    GROUPS[(ti, b)] = [0, NCH]

            def _mm(ti, b, lhsT, pst, w, rhs):
                st = GROUPS[(ti, b)]
                st[0] += 1
                nc.tensor.matmul(
                    out=pst[0:128, 0:w], lhsT=lhsT, rhs=rhs,
                    start=(st[0] == 1), stop=(st[0] == st[1]),
                )

            def emit_s_mm(b, row_local, mel_src, c0, f32=False):
                rg = SBASE[b] + row_local
                if f32:
                    lhsT = WZ32[:, 128 - rg : 256 - rg]
                    rhs = mel_src[:, c0 + 1 : c0 + 261]
                else:
                    lhsT = WZb[:, 128 - rg : 256 - rg]
                    rhs = mel_src[:, c0 + 1 : c0 + 261]
                _mm(0, b, lhsT, psS[b], 260, rhs)

            def emit_qd_mms(b, row_local, sq, cr, c0):
                rg = SBASE[b] + row_local
                lhsT = WZb[:, 128 - rg : 256 - rg]
                _mm(1, b, lhsT, psQ[b], 261, sq[:, c0 : c0 + 261])
                _mm(2, b, lhsT, psD[b], 260, cr[:, c0 : c0 + 260])

            def emit_tile(b, j, g0, ng):
                t0 = g0 * CH
                wmel = ng * CH + 6
                src_lo = t0 - 3
                src_hi = min(T, t0 + ng * CH + 3)
                on_sync = j in QSYNC
                if on_sync:
                    melt32 = pmel32.tile([128, wmel], F32R, tag="melt32")
                    nc.sync.dma_start(
                        out=melt32, in_=mel[b, :, src_lo:src_hi].bitcast(F32R)
                    )
                    # S matmuls run straight off the f32 data (f32r view),
                    # filling the PE while the ACT cast runs
                    for k in range(ng):
                        emit_s_mm(b, g0 + k, melt32, k * CH, f32=True)
                    melt = pmel.tile([128, wmel], BF16, tag="melt")
                    half = (ng // 2) * CH + 6
                    nc.scalar.activation(
                        out=melt[:, 0:half],
                        in_=melt32[:, 0:half].bitcast(F32), func=ACTF.Copy,
                    )
                    nc.scalar.activation(
                        out=melt[:, half:wmel],
                        in_=melt32[:, half:wmel].bitcast(F32), func=ACTF.Copy,
                    )
                else:
                    if j == 0:
                        melt = melt_firsts[b]
                    elif j == len(NGS) - 1:
                        melt = melt_lasts[b]
                    else:
                        melt = pmel.tile([128, wmel], BF16, tag="melt")
                    lo_pad = 3 if j == 0 else 0
                    s_lo = src_lo + lo_pad
                    w_real = src_hi - s_lo
                    nc.gpsimd.dma_start(
                        out=melt[:, lo_pad : lo_pad + w_real],
                        in_=mel[b, :, s_lo:src_hi],
                    )
                sq = psq.tile([128, wmel], BF16, tag="sq")
                wcr = ng * CH + 4
                cross = pcross.tile([128, wcr], BF16, tag="cross")
                nc.vector.tensor_tensor(out=sq, in0=melt, in1=melt, op=ALU.mult)
                nc.vector.tensor_tensor(
                    out=cross, in0=melt[:, 0:wcr], in1=melt[:, 1 : wcr + 1],
                    op=ALU.mult,
                )
                if on_sync:
                    for k in range(ng):
                        emit_qd_mms(b, g0 + k, sq, cross, k * CH)
                else:
                    for k in range(ng):
                        emit_s_mm(b, g0 + k, melt, k * CH)
                        emit_qd_mms(b, g0 + k, sq, cross, k * CH)

            # ---------------- shared precompute (emitted mid-stream-0) ----
            def emit_spec_pre():
                # t1sv = w1*validC / (1 + |dspec|)   [NROWS, 260] bf16
                d = pstat.tile([NROWS, 260], F32)
                nc.vector.tensor_tensor(
                    out=d, in0=specH[:, 1:261], in1=specH[:, 0:260],
                    op=ALU.subtract,
                )
                ad = pstat.tile([NROWS, 260], F32)
                nc.scalar.activation(out=ad, in_=d, func=ACTF.Abs)
                a1 = pstat.tile([NROWS, 260], F32)
                nc.vector.tensor_scalar_add(out=a1, in0=ad, scalar1=1.0)
                srec = pstat.tile([NROWS, 260], F32)
                nc.vector.reciprocal_approx_fast(out=srec, in_=a1)
                t1sv = pstat.tile([NROWS, 260], BF16)
                nc.vector.tensor_tensor(out=t1sv, in0=srec, in1=w1vC, op=ALU.mult)
                return t1sv

            def emit_r_pre():
                g05b = pstat.tile([NROWS, 256], BF16)
                nc.vector.tensor_scalar(
                    out=g05b, in0=r, scalar1=0.5, scalar2=None, op0=ALU.is_gt
                )
                rU = pstat.tile([NROWS, 256], F32)
                nc.vector.tensor_scalar(
                    out=rU, in0=r, scalar1=0.5, scalar2=1.0,
                    op0=ALU.add, op1=ALU.min,
                )
                rDA = pstat.tile([NROWS, 256], F32)
                nc.vector.tensor_scalar(
                    out=rDA, in0=r, scalar1=0.5, scalar2=0.0,
                    op0=ALU.subtract, op1=ALU.max,
                )
                return g05b, rU, rDA

            def emit_r_ladder():
                # k = ceil(10*r-5) in (0..5] via compare ladder; rD0 = r-0.1k
                y = pstat.tile([NROWS, 256], F32)
                nc.vector.tensor_scalar(
                    out=y, in0=r, scalar1=10.0, scalar2=5.0,
                    op0=ALU.mult, op1=ALU.subtract,
                )
                cmps = []
                for jth in range(5):
                    c = pstat.tile([NROWS, 256], F32, name=f"cmp{jth}")
                    nc.vector.tensor_scalar(
                        out=c, in0=y, scalar1=float(jth), scalar2=None,
                        op0=ALU.is_gt,
                    )
                    cmps.append(c)
                ka = pstat.tile([NROWS, 256], F32)
                nc.vector.tensor_tensor(out=ka, in0=cmps[0], in1=cmps[1], op=ALU.add)
                kb = pstat.tile([NROWS, 256], F32)
                nc.vector.tensor_tensor(out=kb, in0=cmps[2], in1=cmps[3], op=ALU.add)
                nc.vector.tensor_tensor(out=ka, in0=ka, in1=kb, op=ALU.add)
                nc.vector.tensor_tensor(out=ka, in0=ka, in1=cmps[4], op=ALU.add)
                rD0 = pstat.tile([NROWS, 256], F32)
                nc.vector.scalar_tensor_tensor(
                    out=rD0, in0=ka, scalar=-0.1, in1=r, op0=ALU.mult, op1=ALU.add
                )
                return rD0

            # ---------------- per-sample tail (list of closures) ----------
            def make_tail(b, t1sv, g05b, rU, rDA, rD0_f):
                r0 = SBASE[b]
                sl = slice(r0, r0 + NCH)
                st = {}

                def p1():
                    # temporal: Sb=bf16(psS), W=win5(Sb)
                    st["Sb"] = pstat.tile([NROWS, 260], BF16, name=f"Sb{b}")
                    nc.scalar.activation(
                        out=st["Sb"][sl], in_=psS[b][sl, 0:260], func=ACTF.Copy
                    )
                    st["W"] = pstat.tile([128, 256], BF16, name=f"W{b}")
                    nc.gpsimd.memset(st["W"], 0.0)
                    wa = pstat.tile([NROWS, 258], BF16, name=f"wa{b}")
                    nc.vector.tensor_tensor(
                        out=wa[sl], in0=st["Sb"][sl][:, 0:258],
                        in1=st["Sb"][sl][:, 1:259], op=ALU.add,
                    )
                    nc.vector.tensor_tensor(
                        out=wa[sl][:, 0:256], in0=wa[sl][:, 0:256],
                        in1=wa[sl][:, 2:258], op=ALU.add,
                    )
                    nc.vector.tensor_tensor(
                        out=st["W"][sl], in0=wa[sl][:, 0:256],
                        in1=st["Sb"][sl][:, 4:260], op=ALU.add,
                    )
                    # cos chain start: Q to SBUF (bf16), den2 = Qa*Qb
                    st["Qs"] = pstat.tile([NROWS, 261], BF16, name=f"Qs{b}")
                    nc.scalar.activation(
                        out=st["Qs"][sl], in_=psQ[b][sl, 0:261], func=ACTF.Copy
                    )
                    st["den2"] = pstat.tile([NROWS, 260], BF16, name=f"dn{b}")
                    nc.vector.tensor_tensor(
                        out=st["den2"][sl], in0=st["Qs"][sl][:, 0:260],
                        in1=st["Qs"][sl][:, 1:261], op=ALU.mult,
                    )
                    st["sd"] = pstat.tile([NROWS, 260], F32, name=f"sd{b}")
                    nc.gpsimd.memset(st["sd"], 1.0)
                    nc.scalar.activation(
                        out=st["sd"][sl], in_=st["den2"][sl], func=ACTF.Sqrt,
                        bias=epsT[sl],
                    )

                def p2():
                    # custom-DVE ops misbehave at partition base 64 on HW:
                    # run the reciprocal over the full [0:NROWS] span (base 0)
                    st["rs"] = pstat.tile([NROWS, 260], F32, name=f"rs{b}")
                    nc.vector.reciprocal_approx_fast(
                        out=st["rs"][0:NROWS, :], in_=st["sd"][0:NROWS, :]
                    )
                    st["cos"] = pstat.tile([NROWS, 260], BF16, name=f"cs{b}")
                    nc.vector.tensor_tensor(
                        out=st["cos"][sl], in0=psD[b][sl, 0:260], in1=st["rs"][sl],
                        op=ALU.mult,
                    )
                    st["consH"] = pstat.tile([NROWS, 260], BF16, name=f"ch{b}")
                    nc.vector.scalar_tensor_tensor(
                        out=st["consH"][sl], in0=st["cos"][sl], scalar=float(w0),
                        in1=t1sv[sl], op0=ALU.mult, op1=ALU.add,
                    )
                    # temporal sums via PE: block-indicator lhsT lands the
                    # per-sample sums broadcast onto that sample's partitions
                    st["Wsq"] = pstat.tile([128, 256], BF16, name=f"Wq{b}")
                    nc.gpsimd.memset(st["Wsq"], 0.0)
                    nc.scalar.activation(
                        out=st["Wsq"][sl], in_=st["W"][sl], func=ACTF.Square
                    )
                    bank = psTB if b == 0 else psBB
                    lT = indB[:, b * NROWS : (b + 1) * NROWS]
                    nc.tensor.matmul(
                        out=bank[0:NROWS, 0:256], lhsT=lT,
                        rhs=st["W"][0:128, 0:256], start=True, stop=True,
                    )
                    nc.tensor.matmul(
                        out=bank[0:NROWS, 256:512], lhsT=lT,
                        rhs=st["Wsq"][0:128, 0:256], start=True, stop=True,
                    )

                def p3():
                    # scalar std/threshold chain on this sample's partitions
                    # (psT rows carry the sample sums broadcast per partition)
                    bank = psTB if b == 0 else psBB
                    sx = pstat.tile([128, 2], F32, name=f"sx{b}")
                    scr = pstat.tile([NROWS, 256], F32, name=f"scr{b}")
                    nc.scalar.activation(
                        out=scr[sl], in_=bank[sl, 0:256], func=ACTF.Copy,
                        accum_out=sx[sl, 0:1],
                    )
                    nc.scalar.activation(
                        out=scr[sl], in_=bank[sl, 256:512], func=ACTF.Copy,
                        accum_out=sx[sl, 1:2],
                    )
                    nc.vector.tensor_scalar_mul(
                        out=sx[sl, 0:1], in0=sx[sl, 0:1], scalar1=float(SMSC)
                    )
                    nc.vector.tensor_scalar_mul(
                        out=sx[sl, 1:2], in0=sx[sl, 1:2], scalar1=float(SMSC * SMSC)
                    )
                    sc = pstat.tile([128, 4], F32, name=f"sc{b}")
                    s2 = sc[sl, 0:1]
                    nc.vector.tensor_tensor(
                        out=s2, in0=sx[sl, 0:1], in1=sx[sl, 0:1], op=ALU.mult
                    )
                    nc.vector.tensor_scalar_mul(out=s2, in0=s2, scalar1=1.0 / float(T))
                    var = sc[sl, 1:2]
                    nc.vector.tensor_tensor(
                        out=var, in0=sx[sl, 1:2], in1=s2, op=ALU.subtract
                    )
                    nc.vector.tensor_scalar_mul(
                        out=var, in0=var, scalar1=1.0 / float(T - 1)
                    )
                    std = sc[sl, 2:3]
                    nc.scalar.activation(out=std, in_=var, func=ACTF.Sqrt)
                    # w2t = w2*(1-std); thrH = 0.7-w2t, thrL = 0.4-w2t
                    w2t = sc[sl, 3:4]
                    nc.vector.tensor_scalar(
                        out=w2t, in0=std, scalar1=-1.0, scalar2=1.0,
                        op0=ALU.mult, op1=ALU.add,
                    )
                    nc.vector.tensor_scalar_mul(out=w2t, in0=w2t, scalar1=float(w2))
                    st["thrS"] = pstat.tile([128, 2], F32, name=f"th{b}")
                    nc.vector.tensor_scalar(
                        out=st["thrS"][sl, 0:1], in0=w2t, scalar1=-1.0, scalar2=0.7,
                        op0=ALU.mult, op1=ALU.add,
                    )
                    nc.vector.tensor_scalar(
                        out=st["thrS"][sl, 1:2], in0=w2t, scalar1=-1.0, scalar2=0.4,
                        op0=ALU.mult, op1=ALU.add,
                    )
                    # local-mean chain
                    st["w5"] = pstat.tile([NROWS, 256], BF16, name=f"w5{b}")
                    ca = pstat.tile([NROWS, 258], BF16, name=f"ca{b}")
                    nc.vector.tensor_tensor(
                        out=ca[sl], in0=st["consH"][sl][:, 0:258],
                        in1=st["consH"][sl][:, 1:259], op=ALU.add,
                    )
                    nc.vector.tensor_tensor(
                        out=ca[sl][:, 0:256], in0=ca[sl][:, 0:256],
                        in1=ca[sl][:, 2:258], op=ALU.add,
                    )
                    nc.vector.tensor_tensor(
                        out=st["w5"][sl], in0=ca[sl][:, 0:256],
                        in1=st["consH"][sl][:, 4:260], op=ALU.add,
                    )
                    st["local"] = pstat.tile([NROWS, 256], BF16, name=f"lc{b}")
                    nc.vector.tensor_tensor(
                        out=st["local"][sl], in0=st["w5"][sl], in1=SMb[sl],
                        op=ALU.mult,
                    )

                def p4():
                    # grads branch
                    st["gr"] = pstat.tile([NROWS, 256], BF16, name=f"gr{b}")
                    nc.vector.tensor_tensor(
                        out=st["gr"][sl], in0=st["consH"][sl][:, 2:258],
                        in1=st["consH"][sl][:, 1:257], op=ALU.subtract,
                    )
                    st["gsq"] = pstat.tile([NROWS, 256], BF16, name=f"gq{b}")
                    nc.vector.tensor_tensor(
                        out=st["gsq"][sl], in0=st["gr"][sl], in1=st["gr"][sl],
                        op=ALU.mult,
                    )
                    st["A"] = pstat.tile([NROWS, 256], BF16, name=f"A{b}")
                    nc.vector.tensor_scalar(
                        out=st["A"][sl], in0=st["gsq"][sl], scalar1=th2,
                        scalar2=None, op0=ALU.is_gt,
                    )
                    # compares
                    st["u"] = pstat.tile([NROWS, 256], BF16, name=f"u{b}")
                    nc.vector.tensor_scalar(
                        out=st["u"][sl], in0=st["local"][sl],
                        scalar1=st["thrS"][sl][:, 0:1], scalar2=None, op0=ALU.is_gt,
                    )
                    st["v"] = pstat.tile([NROWS, 256], BF16, name=f"v{b}")
                    nc.vector.tensor_scalar(
                        out=st["v"][sl], in0=st["local"][sl],
                        scalar1=st["thrS"][sl][:, 1:2], scalar2=None, op0=ALU.is_lt,
                    )

                def p4b():
                    if not KDBG:
                        return
                    nc.sync.dma_start(out=dbg["dbg_Sb"][b], in_=st["Sb"][0:NROWS, 0:260])
                    nc.sync.dma_start(out=dbg["dbg_W"][b], in_=st["W"][0:NROWS, 0:256])
                    nc.sync.dma_start(out=dbg["dbg_thr"][b], in_=st["thrS"][0:NROWS, 0:2])
                    nc.sync.dma_start(
                        out=dbg["dbg_local"][b], in_=st["local"][0:NROWS, 0:256]
                    )
                    nc.sync.dma_start(
                        out=dbg["dbg_consH"][b], in_=st["consH"][0:NROWS, 0:260]
                    )
                    nc.sync.dma_start(out=dbg["dbg_A"][b], in_=st["A"][0:NROWS, 0:256])
                    nc.sync.dma_start(out=dbg["dbg_Qs"][b], in_=st["Qs"][0:NROWS, 0:261])
                    nc.sync.dma_start(out=dbg["dbg_cos"][b], in_=st["cos"][0:NROWS, 0:260])
                    nc.sync.dma_start(out=dbg["dbg_t1sv"][b], in_=t1sv[0:NROWS, 0:260])
                    nc.sync.dma_start(
                        out=dbg["dbg_den2"][b], in_=st["den2"][0:NROWS, 0:260]
                    )
                    nc.sync.dma_start(out=dbg["dbg_sd"][b], in_=st["sd"][0:NROWS, 0:260])
                    nc.sync.dma_start(out=dbg["dbg_rs"][b], in_=st["rs"][0:NROWS, 0:260])
                    Dsd = pstat.tile([NROWS, 260], F32, name=f"Dsd{b}")
                    nc.scalar.activation(out=Dsd[sl], in_=psD[b][sl, 0:260], func=ACTF.Copy)
                    nc.sync.dma_start(out=dbg["dbg_Ds"][b], in_=Dsd[0:NROWS, 0:260])

                def p5():
                    # up = v&gate, dn = u&gate ; act0 = g05|A ; masks f32
                    st["up"] = pstat.tile([NROWS, 256], BF16, name=f"up{b}")
                    nc.vector.tensor_tensor(
                        out=st["up"][sl], in0=st["v"][sl], in1=gate01[sl],
                        op=ALU.mult,
                    )
                    st["dn"] = pstat.tile([NROWS, 256], BF16, name=f"dnm{b}")
                    nc.vector.tensor_tensor(
                        out=st["dn"][sl], in0=st["u"][sl], in1=gate01[sl],
                        op=ALU.mult,
                    )
                    st["act0"] = pstat.tile([NROWS, 256], BF16, name=f"a0{b}")
                    nc.vector.tensor_tensor(
                        out=st["act0"][sl], in0=g05b[sl], in1=st["A"][sl],
                        op=ALU.max,
                    )
                    st["nA"] = pstat.tile([NROWS, 256], BF16, name=f"nA{b}")
                    nc.vector.tensor_scalar(
                        out=st["nA"][sl], in0=st["A"][sl], scalar1=-1.0,
                        scalar2=1.0, op0=ALU.mult, op1=ALU.add,
                    )
                    st["ng"] = pstat.tile([NROWS, 256], BF16, name=f"ng{b}")
                    nc.vector.tensor_tensor(
                        out=st["ng"][sl], in0=st["nA"][sl], in1=g05b[sl],
                        op=ALU.mult,
                    )

                def p6():
                    # copy_predicated is also run full-span at base 0 (custom
                    # partition-sensitive op); masks are zeroed outside [sl]
                    mU = pstat.tile([NROWS, 256], F32, name=f"mU{b}")
                    nc.gpsimd.memset(mU, 0.0)
                    nc.vector.tensor_tensor(
                        out=mU[sl], in0=st["up"][sl], in1=st["act0"][sl],
                        op=ALU.mult,
                    )
                    mDA = pstat.tile([NROWS, 256], F32, name=f"mDA{b}")
                    nc.gpsimd.memset(mDA, 0.0)
                    nc.vector.tensor_tensor(
                        out=mDA[sl], in0=st["dn"][sl], in1=st["A"][sl],
                        op=ALU.mult,
                    )
                    mD0 = pstat.tile([NROWS, 256], F32, name=f"mD0{b}")
                    nc.gpsimd.memset(mD0, 0.0)
                    nc.vector.tensor_tensor(
                        out=mD0[sl], in0=st["dn"][sl], in1=st["ng"][sl],
                        op=ALU.mult,
                    )
                    fs = slice(0, NROWS)
                    nc.vector.copy_predicated(
                        out=r[fs], mask=mU[fs].bitcast(mybir.dt.int32), data=rU[fs]
                    )
                    nc.vector.copy_predicated(
                        out=r[fs], mask=mDA[fs].bitcast(mybir.dt.int32), data=rDA[fs]
                    )
                    nc.vector.copy_predicated(
                        out=r[fs], mask=mD0[fs].bitcast(mybir.dt.int32),
                        data=rD0_f[fs],
                    )
                def p7():
                    ob = out[b]
                    eng = nc.scalar if b == 0 else nc.sync
                    eng.dma_start(
                        out=bass.AP(
                            tensor=ob.tensor, offset=ob.offset,
                            ap=[[256, 46], [1, 256]],
                        ),
                        in_=r[r0 : r0 + 46, :],
                    )
                    eng.dma_start(
                        out=bass.AP(
                            tensor=ob.tensor, offset=ob.offset + 256 * 46,
                            ap=[[256, 1], [1, LASTW]],
                        ),
                        in_=r[r0 + 46 : r0 + 47, 0:LASTW],
                    )

                return [p1, p2, p3, p4, p4b, p5, p6, p7]

            # ---------------- emission schedule ----------------
            # stream sample 0, with spec/r precompute interleaved
            g0 = 0
            for j, ng in enumerate(NGS):
                emit_tile(0, j, g0, ng)
                g0 += ng
                if j == 1:
                    t1sv = emit_spec_pre()
                elif j == 2:
                    g05b, rU, rDA = emit_r_pre()
                elif j == 3:
                    rD0_f = emit_r_ladder()

            tail0 = make_tail(0, t1sv, g05b, rU, rDA, rD0_f)
            # stream sample 1 with tail-0 pieces interleaved
            g0 = 0
            for j, ng in enumerate(NGS):
                emit_tile(1, j, g0, ng)
                g0 += ng
                if j >= 1 and tail0:
                    tail0.pop(0)()
            while tail0:
                tail0.pop(0)()

            for p in make_tail(1, t1sv, g05b, rU, rDA, rD0_f):
                p()

    nc.compile()
    return nc


_CACHE = {}


def _get_nc(wbytes):
    if wbytes not in _CACHE:
        w = np.frombuffer(wbytes, np.float32)
        _CACHE[wbytes] = build_nc(float(w[0]), float(w[1]), float(w[2]))
    return _CACHE[wbytes]


def kernel(**inputs):
    mel = np.ascontiguousarray(np.asarray(inputs["mel_features"], np.float32))
    spec = np.ascontiguousarray(np.asarray(inputs["spectral_features"], np.float32))
    init = np.ascontiguousarray(np.asarray(inputs["initial_boundaries"], np.float32))
    sw = np.asarray(inputs["similarity_weights"], np.float32)
    w = _softmax_f32(sw)
    nc = _get_nc(w.tobytes())

    in_maps = []
    for c in range(NCORES):
        s = slice(c * BPC, (c + 1) * BPC)
        in_maps.append(
            {
                "mel_features": np.ascontiguousarray(mel[s]),
                "spectral_features": np.ascontiguousarray(spec[s]),
                "initial_boundaries": np.ascontiguousarray(init[s]),
            }
        )
    res = run_bass_kernel_spmd(nc, in_maps, core_ids=list(range(NCORES)))
    global _LAST_RESULT
    _LAST_RESULT = res
    outs = [np.asarray(res.results[c]["out"], np.float32) for c in range(NCORES)]
    return np.concatenate(outs, axis=0)


_LAST_RESULT = None


if __name__ == "__main__":
    nc = build_nc(1 / 3, 1 / 3, 1 / 3)
    ninst = sum(len(b.instructions) for b in nc.m.functions[0].blocks)
    print("built ok, instructions:", ninst)



# revision 14
# speedup vs baseline: 1.0278x; 1.0278x over previous
"""Trainium2 Bass kernel for nn_AdaptiveBoundaryRefinement_45861660787095.

Self-contained: takes FULL inputs (B=16,M=128,T=12000), shards batch across 8
NeuronCores (2 samples/core), runs a Bass/Tile kernel per core, gathers.

v3 design (v2 tail numerics kept verbatim; front-end rebuilt from trace data):
- mel streams as RAW F32 over two DMA queues (sync HWDGE + gpsimd SWDGE,
  alternating tiles).  The v2 f32->bf16 cast-DMA saturated all 16 SDMA
  engines at only ~280 GB/s read-side; plain f32 streams at the ~358 GB/s
  HBM ceiling and frees the cast entirely.
- S (column sums) matmuls stream the f32 data directly as f32r: with moving
  dim >=256 the PE streams f32r at 1 cycle/row, so no bf16 copy of mel is
  needed anywhere.
- sq = mel^2 moves to the Scalar engine (ACT Square, f32-in/bf16-out,
  1 elem/lane/cycle) - the DVE was the measured critical resource (50.5us
  busy, 54% occ) and TT tops out at 2x regardless of dtype.
- cross = mel[t]*mel[t+1] stays on DVE (f32 1x) except a few big tiles that
  run on GpSimd, balancing all three elementwise engines under the stream.
- Q and D fuse into ONE matmul per chunk: sq and cross live in one combined
  SBUF buffer (cross at column offset CA) and the rhs is a 2-block strided
  AP [[CA,2],[1,261]]; the out AP [[512,2],[1,261]] writes the Q bank and D
  bank of a 2-bank PSUM tile in one pass.  Halves the MM count -> fewer
  fixed overheads and a denser PE stream (keeps the p-state high).
- Per-sample tail (win5 sums, cos, thresholds, closed-form 5-iteration
  refinement via predicated copies) is byte-identical to the validated v2.
- The batch-global early-stop of the reference is a mathematical no-op.
"""

import os
import sys

import numpy as np

_TRN_REPO = "/opt/trn_rl_repo"
if _TRN_REPO not in sys.path:
    sys.path.insert(0, _TRN_REPO)

import concourse.bass as bass
import concourse.bacc as bacc
import concourse.mybir as mybir
import concourse.tile as tile
from concourse.bass_utils import run_bass_kernel_spmd

F32 = mybir.dt.float32
F32R = mybir.dt.float32r
BF16 = mybir.dt.bfloat16
ALU = mybir.AluOpType
ACTF = mybir.ActivationFunctionType
AX = mybir.AxisListType

B, M, T = 16, 128, 12000
NCORES = 8
BPC = B // NCORES            # samples per core = 2
CH = 256                     # chunk width
NCH = (T + CH - 1) // CH     # 47 chunks per sample
EPS2 = 1e-12
GRAD_THRESH = 0.15
LASTW = T - CH * (NCH - 1)   # 224 real cols in the last chunk
NGS = [1, 4, 8, 8, 8, 8, 8, 2]  # chunks per mel tile (small ramp/tail tiles)
SBASE = (0, 64)              # partition base of each sample's chunk rows
NROWS = SBASE[1] + NCH       # 111
CA = 2056                    # cross region offset inside the combined qd tile
QDW = CA + 2053 + 3          # combined sq|cross tile width (bf16)
# which (sample, tile) crosses run on GpSimd instead of DVE: the small early
# tiles, while the Q7 has nothing else queued
GPS_CROSS = {(0, 0), (0, 1), (1, 0), (1, 1)}


def _softmax_f32(x):
    x = np.asarray(x, np.float32)
    m = np.max(x).astype(np.float32)
    e = np.exp((x - m).astype(np.float32)).astype(np.float32)
    return (e / e.sum(dtype=np.float32).astype(np.float32)).astype(np.float32)


def _const_masks(w0, w1):
    import ml_dtypes

    bf = ml_dtypes.bfloat16
    # w1vC [NROWS, 260]: w1 where col maps to t in [0, T), else 0
    # (col h of chunk p -> t = 256p - 2 + h).  Also t=0 keeps w1 (spec_sim=1
    # there via the specH shift trick).
    w1vC = np.full((NROWS, 260), np.float32(w1), np.float32)
    for r0 in SBASE:
        w1vC[r0, 0:2] = 0.0                       # t < 0
        w1vC[r0 + NCH - 1, LASTW + 2 : 260] = 0.0  # t >= T
    # SMb [NROWS, 256]: 0.2 interior, 0.25 at t=1 and t=T-2 (win4 edges)
    SM = np.full((NROWS, 256), np.float32(0.2), np.float32)
    for r0 in SBASE:
        SM[r0, 1] = 0.25
        SM[r0 + NCH - 1, LASTW - 2] = 0.25
    # gate01 [NROWS, 256]: 1 for t in [1, T-2], else 0
    gate = np.ones((NROWS, 256), np.float32)
    for r0 in SBASE:
        gate[r0, 0] = 0.0
        gate[r0 + NCH - 1, LASTW - 1 : 256] = 0.0
    # wzb one-hot bank [128, 257]: col 128 is ones; slice [128-j : 256-j]
    # puts the hot column at position j (j in 0..127).
    wzb = np.zeros((128, 257), bf)
    wzb[:, 128] = 1.0
    wz32 = np.zeros((128, 257), np.float32)
    wz32[:, 128] = 1.0
    # indB [128, 2*NROWS]: cols [0:NROWS] sample-0 block-indicator lhsT,
    # cols [NROWS:2*NROWS] sample-1.  out[i,:] = sum_{p in sample} W[p,:]
    # lands on every row i of that sample -> per-partition broadcast sums.
    indB = np.zeros((128, 2 * NROWS), bf)
    indB[0:NCH, 0:NCH] = 1.0
    indB[SBASE[1] : SBASE[1] + NCH, NROWS + SBASE[1] : NROWS + SBASE[1] + NCH] = 1.0
    return w1vC.astype(bf), SM.astype(bf), gate.astype(bf), wzb, indB, wz32


def build_nc(w0, w1, w2):
    nc = bacc.Bacc("TRN2", target_bir_lowering=False, debug=False)
    mel = nc.dram_tensor("mel_features", [BPC, M, T], F32, kind="ExternalInput")
    spec = nc.dram_tensor("spectral_features", [BPC, T], F32, kind="ExternalInput")
    init = nc.dram_tensor("initial_boundaries", [BPC, T], F32, kind="ExternalInput")
    out = nc.dram_tensor("out", [BPC, T], F32, kind="ExternalOutput")
    KDBG = bool(os.environ.get("KDBG"))
    dbg = {}
    if KDBG:
        for nm, w, dt in (
            ("dbg_W", 256, BF16), ("dbg_thr", 2, F32), ("dbg_local", 256, BF16),
            ("dbg_consH", 260, BF16), ("dbg_A", 256, BF16), ("dbg_Sb", 260, BF16),
            ("dbg_Qs", 261, BF16), ("dbg_cos", 260, BF16), ("dbg_t1sv", 260, BF16),
            ("dbg_Ds", 260, F32), ("dbg_den2", 260, BF16), ("dbg_sd", 260, F32),
            ("dbg_rs", 260, F32),
        ):
            dbg[nm] = nc.dram_tensor(nm, [BPC, NROWS, w], dt, kind="ExternalOutput")

    w1vC_np, SM_np, gate_np, wzb_np, indB_np, wz32_np = _const_masks(w0, w1)
    w1vC_d = nc.inline_tensor(w1vC_np, name="w1vC")
    wzb_d = nc.inline_tensor(wzb_np, name="wzbc")
    wz32_d = nc.inline_tensor(wz32_np, name="wz32c")
    indB_d = nc.inline_tensor(indB_np, name="indB")
    SM_d = nc.inline_tensor(SM_np, name="SMb")
    gate_d = nc.inline_tensor(gate_np, name="gate01")

    th2 = float(np.float32(GRAD_THRESH) * np.float32(GRAD_THRESH))
    SMSC = 0.2 / 128.0

    with tile.TileContext(nc) as tc:
        with (
            tc.tile_pool(name="mel", bufs=8) as pmel,
            tc.tile_pool(name="qd", bufs=5) as pqd,
            tc.tile_pool(name="stat", bufs=1) as pstat,
            tc.tile_pool(name="ps", bufs=1, space="PSUM") as pps,
            tc.tile_pool(name="ps2", bufs=1, space="PSUM") as pps2,
        ):
            # ---------------- persistent first/last mel tiles (f32) -------
            W_FIRST = NGS[0] * CH + 6
            W_LAST = NGS[-1] * CH + 6
            T0_LAST = (NCH - NGS[-1]) * CH
            LASTREAL = T - (T0_LAST - 3)
            melt_firsts, melt_lasts = [], []
            for bb in range(BPC):
                mf = pstat.tile([128, W_FIRST], F32R, name=f"mf{bb}")
                melt_firsts.append(mf)
                ml = pstat.tile([128, W_LAST], F32R, name=f"ml{bb}")
                melt_lasts.append(ml)

            # constants ride the sync (HWDGE) queue ahead of the first mel
            # tile: tiny bytes, and the idle SP sequencer absorbs the per-DMA
            # issue cost (GpSimd Q7 descgen is ~670ns each and would delay
            # the odd-tile mel stream; ACT must stay DMA-free for sq).
            WZb = pstat.tile([128, 257], BF16)
            nc.sync.dma_start(out=WZb, in_=wzb_d[:, :])
            WZ32 = pstat.tile([128, 257], F32R)
            nc.sync.dma_start(out=WZ32, in_=wz32_d[:, :].bitcast(F32R))

            # first mel tile DMA goes out right after the one-hot banks
            nc.sync.dma_start(
                out=melt_firsts[0][:, 3 : 3 + CH + 3],
                in_=mel[0, :, 0 : CH + 3].bitcast(F32R),
            )

            indB = pstat.tile([128, 2 * NROWS], BF16)
            nc.sync.dma_start(out=indB, in_=indB_d[:, :])
            w1vC = pstat.tile([NROWS, 260], BF16)
            nc.sync.dma_start(out=w1vC, in_=w1vC_d[:, :])
            SMb = pstat.tile([NROWS, 256], BF16)
            nc.sync.dma_start(out=SMb, in_=SM_d[:, :])
            gate01 = pstat.tile([NROWS, 256], BF16)
            nc.sync.dma_start(out=gate01, in_=gate_d[:, :])

            # dummy Sqrt: forces the sqrt_and_others table
            # (square/abs/copy/identity/sqrt) once, at t~0 - no mid-kernel
            # ACT table reload.  Square (the sq op) lives in the same set.
            dummy = pstat.tile([1, 1], F32)
            nc.vector.memset(dummy, 1.0)
            nc.scalar.activation(out=dummy, in_=dummy, func=ACTF.Sqrt)
            epsT = pstat.tile([128, 1], F32)
            nc.vector.memset(epsT, float(EPS2))

            # zero the persistent tile pads (f32 views)
            for bb in range(BPC):
                nc.vector.memset(melt_firsts[bb][:, 0:3].bitcast(F32), 0.0)
                nc.vector.memset(
                    melt_lasts[bb][:, LASTREAL:W_LAST].bitcast(F32), 0.0
                )

            specH = pstat.tile([NROWS, 261], F32)  # spec at t = 256p-3+h
            nc.vector.memset(specH, 0.0)
            r = pstat.tile([NROWS, 256], F32)
            nc.vector.memset(r, 0.0)

            def _dma_rows(eng, dst, src_1d, row_lo, row_hi, col_off, width, t_base):
                ap = bass.AP(
                    tensor=src_1d.tensor,
                    offset=src_1d.offset + t_base,
                    ap=[[256, row_hi - row_lo], [1, width]],
                )
                eng.dma_start(
                    out=dst[row_lo:row_hi, col_off : col_off + width], in_=ap
                )

            def emit_smalls(b):
                r0 = SBASE[b]
                sp = spec[b]
                _dma_rows(nc.sync, specH, sp, r0, r0 + 1, 3, 258, 0)
                _dma_rows(nc.sync, specH, sp, r0, r0 + 1, 2, 1, 0)
                _dma_rows(nc.sync, specH, sp, r0 + 1, r0 + 46, 0, 261, 256 - 3)
                _dma_rows(nc.sync, specH, sp, r0 + 46, r0 + 47, 0, 227,
                          256 * 46 - 3)
                ini = init[b]
                _dma_rows(nc.sync, r, ini, r0, r0 + 46, 0, 256, 0)
                _dma_rows(nc.sync, r, ini, r0 + 46, r0 + 47, 0, LASTW, 256 * 46)

            # ---------------- PSUM ----------------
            # psS_b  [128, 512]: col sums,    S[rg, h], h -> t = 256rg-2+h
            # psQD_b [128,1024]: bank0 Q[rg, 0:261] (t=256rg-3+h),
            #                    bank1 D[rg, 0:260] at cols 512:772
            psS = [pps.tile([128, 512], F32, name=f"psS{b}") for b in range(BPC)]
            psQD = [pps.tile([128, 1024], F32, name=f"psQD{b}") for b in range(BPC)]
            # two extra banks hold the tiny temporal/broadcast outputs
            psTB = pps2.tile([128, 512], F32)
            psBB = pps2.tile([128, 512], F32)

            def _mm(ti, b, row_local, bank, lhsT_onehot, cols, rhs):
                rg = SBASE[b] + row_local
                lhsT = lhsT_onehot[:, 128 - rg : 256 - rg]
                nc.tensor.matmul(
                    out=bank[0:128, cols], lhsT=lhsT, rhs=rhs,
                    start=(row_local == 0),
                    stop=(row_local == NCH - 1),
                )

            def emit_s_mm(b, row_local, melt, c0):
                _mm(0, b, row_local, psS[b], WZ32, slice(0, 260),
                    melt[:, c0 + 1 : c0 + 261])

            def emit_qd_mm(b, row_local, qd, c0):
                _mm(1, b, row_local, psQD[b], WZb, slice(0, 261),
                    qd[:, c0 : c0 + 261])
                _mm(2, b, row_local, psQD[b], WZb, slice(512, 772),
                    qd[:, CA + c0 : CA + c0 + 260])

            def emit_tile(b, j, g0, ng):
                t0 = g0 * CH
                wmel = ng * CH + 6
                src_lo = t0 - 3
                src_hi = min(T, t0 + ng * CH + 3)
                if j == 0:
                    melt = melt_firsts[b]
                elif j == len(NGS) - 1:
                    melt = melt_lasts[b]
                else:
                    melt = pmel.tile([128, wmel], F32R, tag="melt")
                lo_pad = 3 if j == 0 else 0
                s_lo = src_lo + lo_pad
                w_real = src_hi - s_lo
                if not (b == 0 and j == 0):  # (0,0) already issued up top
                    eng = nc.sync if (j % 2 == 0) else nc.gpsimd
                    eng.dma_start(
                        out=melt[:, lo_pad : lo_pad + w_real],
                        in_=mel[b, :, s_lo:src_hi].bitcast(F32R),
                    )
                # S matmuls run straight off the f32 data (f32r stream)
                for k in range(ng):
                    emit_s_mm(b, g0 + k, melt, k * CH)
                # combined sq|cross buffer (bf16): sq at [0,wmel),
                # cross at [CA, CA+wmel-1)
                qd = pqd.tile([128, QDW], BF16, tag="qd")
                nc.scalar.activation(
                    out=qd[:, 0:wmel], in_=melt[:, 0:wmel].bitcast(F32),
                    func=ACTF.Square,
                )
                wx = wmel - 1
                ceng = nc.gpsimd if (b, j) in GPS_CROSS else nc.vector
                ceng.tensor_tensor(
                    out=qd[:, CA : CA + wx],
                    in0=melt[:, 0:wx].bitcast(F32),
                    in1=melt[:, 1 : wx + 1].bitcast(F32), op=ALU.mult,
                )
                for k in range(ng):
                    emit_qd_mm(b, g0 + k, qd, k * CH)

            # ---------------- shared precompute (emitted mid-stream-0) ----
            def emit_spec_pre():
                # t1sv = w1*validC / (1 + |dspec|)   [NROWS, 260] bf16
                d = pstat.tile([NROWS, 260], F32)
                nc.vector.tensor_tensor(
                    out=d, in0=specH[:, 1:261], in1=specH[:, 0:260],
                    op=ALU.subtract,
                )
                ad = pstat.tile([NROWS, 260], F32)
                nc.scalar.activation(out=ad, in_=d, func=ACTF.Abs)
                a1 = pstat.tile([NROWS, 260], F32)
                nc.vector.tensor_scalar_add(out=a1, in0=ad, scalar1=1.0)
                srec = pstat.tile([NROWS, 260], F32)
                nc.vector.reciprocal_approx_fast(out=srec, in_=a1)
                t1sv = pstat.tile([NROWS, 260], BF16)
                nc.vector.tensor_tensor(out=t1sv, in0=srec, in1=w1vC, op=ALU.mult)
                return t1sv

            def emit_r_pre():
                # on GpSimd: the Q7 is idle mid-stream and this keeps the
                # DVE free for cross
                g05b = pstat.tile([NROWS, 256], BF16)
                nc.vector.tensor_scalar(
                    out=g05b, in0=r, scalar1=0.5, scalar2=None, op0=ALU.is_gt
                )
                rU = pstat.tile([NROWS, 256], F32)
                nc.vector.tensor_scalar(
                    out=rU, in0=r, scalar1=0.5, scalar2=1.0,
                    op0=ALU.add, op1=ALU.min,
                )
                rDA = pstat.tile([NROWS, 256], F32)
                nc.vector.tensor_scalar(
                    out=rDA, in0=r, scalar1=0.5, scalar2=0.0,
                    op0=ALU.subtract, op1=ALU.max,
                )
                return g05b, rU, rDA

            def emit_r_ladder():
                # k = ceil(10*r-5) in (0..5] via compare ladder; rD0 = r-0.1k
                # (also on GpSimd, see emit_r_pre)
                y = pstat.tile([NROWS, 256], F32)
                nc.vector.tensor_scalar(
                    out=y, in0=r, scalar1=10.0, scalar2=5.0,
                    op0=ALU.mult, op1=ALU.subtract,
                )
                cmps = []
                for jth in range(5):
                    c = pstat.tile([NROWS, 256], F32, name=f"cmp{jth}")
                    nc.vector.tensor_scalar(
                        out=c, in0=y, scalar1=float(jth), scalar2=None,
                        op0=ALU.is_gt,
                    )
                    cmps.append(c)
                ka = pstat.tile([NROWS, 256], F32)
                nc.gpsimd.tensor_tensor(out=ka, in0=cmps[0], in1=cmps[1], op=ALU.add)
                kb = pstat.tile([NROWS, 256], F32)
                nc.gpsimd.tensor_tensor(out=kb, in0=cmps[2], in1=cmps[3], op=ALU.add)
                nc.gpsimd.tensor_tensor(out=ka, in0=ka, in1=kb, op=ALU.add)
                nc.gpsimd.tensor_tensor(out=ka, in0=ka, in1=cmps[4], op=ALU.add)
                rD0 = pstat.tile([NROWS, 256], F32)
                nc.vector.scalar_tensor_tensor(
                    out=rD0, in0=ka, scalar=-0.1, in1=r, op0=ALU.mult, op1=ALU.add
                )
                return rD0

            # ---------------- per-sample tail (list of closures) ----------
            def make_tail(b, t1sv, g05b, rU, rDA, rD0_f):
                r0 = SBASE[b]
                sl = slice(r0, r0 + NCH)
                st = {}

                def p1():
                    # temporal: Sb=bf16(psS), W=win5(Sb)
                    st["Sb"] = pstat.tile([NROWS, 260], BF16, name=f"Sb{b}")
                    nc.scalar.activation(
                        out=st["Sb"][sl], in_=psS[b][sl, 0:260], func=ACTF.Copy
                    )
                    st["W"] = pstat.tile([128, 256], BF16, name=f"W{b}")
                    nc.gpsimd.memset(st["W"], 0.0)
                    wa = pstat.tile([NROWS, 258], BF16, name=f"wa{b}")
                    nc.vector.tensor_tensor(
                        out=wa[sl], in0=st["Sb"][sl][:, 0:258],
                        in1=st["Sb"][sl][:, 1:259], op=ALU.add,
                    )
                    nc.vector.tensor_tensor(
                        out=wa[sl][:, 0:256], in0=wa[sl][:, 0:256],
                        in1=wa[sl][:, 2:258], op=ALU.add,
                    )
                    nc.vector.tensor_tensor(
                        out=st["W"][sl], in0=wa[sl][:, 0:256],
                        in1=st["Sb"][sl][:, 4:260], op=ALU.add,
                    )
                    # cos chain start: Q to SBUF (bf16), den2 = Qa*Qb
                    st["Qs"] = pstat.tile([NROWS, 261], BF16, name=f"Qs{b}")
                    nc.scalar.activation(
                        out=st["Qs"][sl], in_=psQD[b][sl, 0:261], func=ACTF.Copy
                    )
                    st["den2"] = pstat.tile([NROWS, 260], BF16, name=f"dn{b}")
                    nc.vector.tensor_tensor(
                        out=st["den2"][sl], in0=st["Qs"][sl][:, 0:260],
                        in1=st["Qs"][sl][:, 1:261], op=ALU.mult,
                    )
                    st["sd"] = pstat.tile([NROWS, 260], F32, name=f"sd{b}")
                    nc.gpsimd.memset(st["sd"], 1.0)
                    nc.scalar.activation(
                        out=st["sd"][sl], in_=st["den2"][sl], func=ACTF.Sqrt,
                        bias=epsT[sl],
                    )

                def p2():
                    # custom-DVE ops misbehave at partition base 64 on HW:
                    # run the reciprocal over the full [0:NROWS] span (base 0)
                    st["rs"] = pstat.tile([NROWS, 260], F32, name=f"rs{b}")
                    nc.vector.reciprocal_approx_fast(
                        out=st["rs"][0:NROWS, :], in_=st["sd"][0:NROWS, :]
                    )
                    st["cos"] = pstat.tile([NROWS, 260], BF16, name=f"cs{b}")
                    nc.vector.tensor_tensor(
                        out=st["cos"][sl], in0=psQD[b][sl, 512:772],
                        in1=st["rs"][sl], op=ALU.mult,
                    )
                    st["consH"] = pstat.tile([NROWS, 260], BF16, name=f"ch{b}")
                    nc.vector.scalar_tensor_tensor(
                        out=st["consH"][sl], in0=st["cos"][sl], scalar=float(w0),
                        in1=t1sv[sl], op0=ALU.mult, op1=ALU.add,
                    )
                    # temporal sums via PE: block-indicator lhsT lands the
                    # per-sample sums broadcast onto that sample's partitions
                    st["Wsq"] = pstat.tile([128, 256], BF16, name=f"Wq{b}")
                    nc.gpsimd.memset(st["Wsq"], 0.0)
                    nc.scalar.activation(
                        out=st["Wsq"][sl], in_=st["W"][sl], func=ACTF.Square
                    )
                    bank = psTB if b == 0 else psBB
                    lT = indB[:, b * NROWS : (b + 1) * NROWS]
                    nc.tensor.matmul(
                        out=bank[0:NROWS, 0:256], lhsT=lT,
                        rhs=st["W"][0:128, 0:256], start=True, stop=True,
                    )
                    nc.tensor.matmul(
                        out=bank[0:NROWS, 256:512], lhsT=lT,
                        rhs=st["Wsq"][0:128, 0:256], start=True, stop=True,
                    )

                def p3():
                    # scalar std/threshold chain on this sample's partitions
                    # (psT rows carry the sample sums broadcast per partition)
                    bank = psTB if b == 0 else psBB
                    sx = pstat.tile([128, 2], F32, name=f"sx{b}")
                    scr = pstat.tile([NROWS, 256], F32, name=f"scr{b}")
                    nc.scalar.activation(
                        out=scr[sl], in_=bank[sl, 0:256], func=ACTF.Copy,
                        accum_out=sx[sl, 0:1],
                    )
                    nc.scalar.activation(
                        out=scr[sl], in_=bank[sl, 256:512], func=ACTF.Copy,
                        accum_out=sx[sl, 1:2],
                    )
                    nc.vector.tensor_scalar_mul(
                        out=sx[sl, 0:1], in0=sx[sl, 0:1], scalar1=float(SMSC)
                    )
                    nc.vector.tensor_scalar_mul(
                        out=sx[sl, 1:2], in0=sx[sl, 1:2], scalar1=float(SMSC * SMSC)
                    )
                    sc = pstat.tile([128, 4], F32, name=f"sc{b}")
                    s2 = sc[sl, 0:1]
                    nc.vector.tensor_tensor(
                        out=s2, in0=sx[sl, 0:1], in1=sx[sl, 0:1], op=ALU.mult
                    )
                    nc.vector.tensor_scalar_mul(out=s2, in0=s2, scalar1=1.0 / float(T))
                    var = sc[sl, 1:2]
                    nc.vector.tensor_tensor(
                        out=var, in0=sx[sl, 1:2], in1=s2, op=ALU.subtract
                    )
                    nc.vector.tensor_scalar_mul(
                        out=var, in0=var, scalar1=1.0 / float(T - 1)
                    )
                    std = sc[sl, 2:3]
                    nc.scalar.activation(out=std, in_=var, func=ACTF.Sqrt)
                    # w2t = w2*(1-std); thrH = 0.7-w2t, thrL = 0.4-w2t
                    w2t = sc[sl, 3:4]
                    nc.vector.tensor_scalar(
                        out=w2t, in0=std, scalar1=-1.0, scalar2=1.0,
                        op0=ALU.mult, op1=ALU.add,
                    )
                    nc.vector.tensor_scalar_mul(out=w2t, in0=w2t, scalar1=float(w2))
                    st["thrS"] = pstat.tile([128, 2], F32, name=f"th{b}")
                    nc.vector.tensor_scalar(
                        out=st["thrS"][sl, 0:1], in0=w2t, scalar1=-1.0, scalar2=0.7,
                        op0=ALU.mult, op1=ALU.add,
                    )
                    nc.vector.tensor_scalar(
                        out=st["thrS"][sl, 1:2], in0=w2t, scalar1=-1.0, scalar2=0.4,
                        op0=ALU.mult, op1=ALU.add,
                    )
                    # local-mean chain
                    st["w5"] = pstat.tile([NROWS, 256], BF16, name=f"w5{b}")
                    ca = pstat.tile([NROWS, 258], BF16, name=f"ca{b}")
                    nc.vector.tensor_tensor(
                        out=ca[sl], in0=st["consH"][sl][:, 0:258],
                        in1=st["consH"][sl][:, 1:259], op=ALU.add,
                    )
                    nc.vector.tensor_tensor(
                        out=ca[sl][:, 0:256], in0=ca[sl][:, 0:256],
                        in1=ca[sl][:, 2:258], op=ALU.add,
                    )
                    nc.vector.tensor_tensor(
                        out=st["w5"][sl], in0=ca[sl][:, 0:256],
                        in1=st["consH"][sl][:, 4:260], op=ALU.add,
                    )
                    st["local"] = pstat.tile([NROWS, 256], BF16, name=f"lc{b}")
                    nc.vector.tensor_tensor(
                        out=st["local"][sl], in0=st["w5"][sl], in1=SMb[sl],
                        op=ALU.mult,
                    )

                def p4():
                    # grads branch
                    st["gr"] = pstat.tile([NROWS, 256], BF16, name=f"gr{b}")
                    nc.vector.tensor_tensor(
                        out=st["gr"][sl], in0=st["consH"][sl][:, 2:258],
                        in1=st["consH"][sl][:, 1:257], op=ALU.subtract,
                    )
                    st["gsq"] = pstat.tile([NROWS, 256], BF16, name=f"gq{b}")
                    nc.vector.tensor_tensor(
                        out=st["gsq"][sl], in0=st["gr"][sl], in1=st["gr"][sl],
                        op=ALU.mult,
                    )
                    st["A"] = pstat.tile([NROWS, 256], BF16, name=f"A{b}")
                    nc.vector.tensor_scalar(
                        out=st["A"][sl], in0=st["gsq"][sl], scalar1=th2,
                        scalar2=None, op0=ALU.is_gt,
                    )
                    # compares
                    st["u"] = pstat.tile([NROWS, 256], BF16, name=f"u{b}")
                    nc.vector.tensor_scalar(
                        out=st["u"][sl], in0=st["local"][sl],
                        scalar1=st["thrS"][sl][:, 0:1], scalar2=None, op0=ALU.is_gt,
                    )
                    st["v"] = pstat.tile([NROWS, 256], BF16, name=f"v{b}")
                    nc.vector.tensor_scalar(
                        out=st["v"][sl], in0=st["local"][sl],
                        scalar1=st["thrS"][sl][:, 1:2], scalar2=None, op0=ALU.is_lt,
                    )

                def p4b():
                    if not KDBG:
                        return
                    nc.sync.dma_start(out=dbg["dbg_Sb"][b], in_=st["Sb"][0:NROWS, 0:260])
                    nc.sync.dma_start(out=dbg["dbg_W"][b], in_=st["W"][0:NROWS, 0:256])
                    nc.sync.dma_start(out=dbg["dbg_thr"][b], in_=st["thrS"][0:NROWS, 0:2])
                    nc.sync.dma_start(
                        out=dbg["dbg_local"][b], in_=st["local"][0:NROWS, 0:256]
                    )
                    nc.sync.dma_start(
                        out=dbg["dbg_consH"][b], in_=st["consH"][0:NROWS, 0:260]
                    )
                    nc.sync.dma_start(out=dbg["dbg_A"][b], in_=st["A"][0:NROWS, 0:256])
                    nc.sync.dma_start(out=dbg["dbg_Qs"][b], in_=st["Qs"][0:NROWS, 0:261])
                    nc.sync.dma_start(out=dbg["dbg_cos"][b], in_=st["cos"][0:NROWS, 0:260])
                    nc.sync.dma_start(out=dbg["dbg_t1sv"][b], in_=t1sv[0:NROWS, 0:260])
                    nc.sync.dma_start(
                        out=dbg["dbg_den2"][b], in_=st["den2"][0:NROWS, 0:260]
                    )
                    nc.sync.dma_start(out=dbg["dbg_sd"][b], in_=st["sd"][0:NROWS, 0:260])
                    nc.sync.dma_start(out=dbg["dbg_rs"][b], in_=st["rs"][0:NROWS, 0:260])
                    Dsd = pstat.tile([NROWS, 260], F32, name=f"Dsd{b}")
                    nc.scalar.activation(
                        out=Dsd[sl], in_=psQD[b][sl, 512:772], func=ACTF.Copy
                    )
                    nc.sync.dma_start(out=dbg["dbg_Ds"][b], in_=Dsd[0:NROWS, 0:260])

                def p5():
                    # up = v&gate, dn = u&gate ; act0 = g05|A ; masks f32
                    st["up"] = pstat.tile([NROWS, 256], BF16, name=f"up{b}")
                    nc.vector.tensor_tensor(
                        out=st["up"][sl], in0=st["v"][sl], in1=gate01[sl],
                        op=ALU.mult,
                    )
                    st["dn"] = pstat.tile([NROWS, 256], BF16, name=f"dnm{b}")
                    nc.vector.tensor_tensor(
                        out=st["dn"][sl], in0=st["u"][sl], in1=gate01[sl],
                        op=ALU.mult,
                    )
                    st["act0"] = pstat.tile([NROWS, 256], BF16, name=f"a0{b}")
                    nc.vector.tensor_tensor(
                        out=st["act0"][sl], in0=g05b[sl], in1=st["A"][sl],
                        op=ALU.max,
                    )
                    st["nA"] = pstat.tile([NROWS, 256], BF16, name=f"nA{b}")
                    nc.vector.tensor_scalar(
                        out=st["nA"][sl], in0=st["A"][sl], scalar1=-1.0,
                        scalar2=1.0, op0=ALU.mult, op1=ALU.add,
                    )
                    st["ng"] = pstat.tile([NROWS, 256], BF16, name=f"ng{b}")
                    nc.vector.tensor_tensor(
                        out=st["ng"][sl], in0=st["nA"][sl], in1=g05b[sl],
                        op=ALU.mult,
                    )

                def p6():
                    # copy_predicated is also run full-span at base 0 (custom
                    # partition-sensitive op); masks are zeroed outside [sl]
                    mU = pstat.tile([NROWS, 256], F32, name=f"mU{b}")
                    nc.gpsimd.memset(mU, 0.0)
                    nc.vector.tensor_tensor(
                        out=mU[sl], in0=st["up"][sl], in1=st["act0"][sl],
                        op=ALU.mult,
                    )
                    mDA = pstat.tile([NROWS, 256], F32, name=f"mDA{b}")
                    nc.gpsimd.memset(mDA, 0.0)
                    nc.vector.tensor_tensor(
                        out=mDA[sl], in0=st["dn"][sl], in1=st["A"][sl],
                        op=ALU.mult,
                    )
                    mD0 = pstat.tile([NROWS, 256], F32, name=f"mD0{b}")
                    nc.gpsimd.memset(mD0, 0.0)
                    nc.vector.tensor_tensor(
                        out=mD0[sl], in0=st["dn"][sl], in1=st["ng"][sl],
                        op=ALU.mult,
                    )
                    fs = slice(0, NROWS)
                    nc.vector.copy_predicated(
                        out=r[fs], mask=mU[fs].bitcast(mybir.dt.int32), data=rU[fs]
                    )
                    nc.vector.copy_predicated(
                        out=r[fs], mask=mDA[fs].bitcast(mybir.dt.int32), data=rDA[fs]
                    )
                    nc.vector.copy_predicated(
                        out=r[fs], mask=mD0[fs].bitcast(mybir.dt.int32),
                        data=rD0_f[fs],
                    )

                def p7():
                    ob = out[b]
                    eng = nc.scalar if b == 0 else nc.sync
                    eng.dma_start(
                        out=bass.AP(
                            tensor=ob.tensor, offset=ob.offset,
                            ap=[[256, 46], [1, 256]],
                        ),
                        in_=r[r0 : r0 + 46, :],
                    )
                    eng.dma_start(
                        out=bass.AP(
                            tensor=ob.tensor, offset=ob.offset + 256 * 46,
                            ap=[[256, 1], [1, LASTW]],
                        ),
                        in_=r[r0 + 46 : r0 + 47, 0:LASTW],
                    )

                return [p1, p2, p3, p4, p4b, p5, p6, p7]

            # ---------------- emission schedule ----------------
            # stream sample 0, with spec/r precompute interleaved
            g0 = 0
            for j, ng in enumerate(NGS):
                emit_tile(0, j, g0, ng)
                g0 += ng
                if j == 1:
                    emit_smalls(0)
                    emit_smalls(1)
                elif j == 2:
                    t1sv = emit_spec_pre()
                    g05b, rU, rDA = emit_r_pre()
                elif j == 3:
                    rD0_f = emit_r_ladder()

            tail0 = make_tail(0, t1sv, g05b, rU, rDA, rD0_f)
            # stream sample 1 with tail-0 pieces interleaved
            g0 = 0
            for j, ng in enumerate(NGS):
                emit_tile(1, j, g0, ng)
                g0 += ng
                if j >= 1 and tail0:
                    tail0.pop(0)()
            while tail0:
                tail0.pop(0)()

            for p in make_tail(1, t1sv, g05b, rU, rDA, rD0_f):
                p()

    nc.compile()
    return nc


_CACHE = {}


def _get_nc(wbytes):
    if wbytes not in _CACHE:
        w = np.frombuffer(wbytes, np.float32)
        _CACHE[wbytes] = build_nc(float(w[0]), float(w[1]), float(w[2]))
    return _CACHE[wbytes]


def kernel(**inputs):
    mel = np.ascontiguousarray(np.asarray(inputs["mel_features"], np.float32))
    spec = np.ascontiguousarray(np.asarray(inputs["spectral_features"], np.float32))
    init = np.ascontiguousarray(np.asarray(inputs["initial_boundaries"], np.float32))
    sw = np.asarray(inputs["similarity_weights"], np.float32)
    w = _softmax_f32(sw)
    nc = _get_nc(w.tobytes())

    in_maps = []
    for c in range(NCORES):
        s = slice(c * BPC, (c + 1) * BPC)
        in_maps.append(
            {
                "mel_features": np.ascontiguousarray(mel[s]),
                "spectral_features": np.ascontiguousarray(spec[s]),
                "initial_boundaries": np.ascontiguousarray(init[s]),
            }
        )
    res = run_bass_kernel_spmd(nc, in_maps, core_ids=list(range(NCORES)))
    global _LAST_RESULT
    _LAST_RESULT = res
    outs = [np.asarray(res.results[c]["out"], np.float32) for c in range(NCORES)]
    return np.concatenate(outs, axis=0)


_LAST_RESULT = None


if __name__ == "__main__":
    nc = build_nc(1 / 3, 1 / 3, 1 / 3)
    ninst = sum(len(b.instructions) for b in nc.m.functions[0].blocks)
    print("built ok, instructions:", ninst)
